# revision 2
# baseline (speedup 1.0000x reference)
"""Trainium2 Bass kernel for nn_NeuralNet_19250043421419.

Row-normalize x (mean/std over D=3072, ddof=1) then a 3-layer MLP
(3072->32->32->10) with LeakyReLU(0.01) after every layer.

Strategy: pure data parallel over 8 NeuronCores (batch 32768 -> 4096/core).
Per core, per 512-row block:
  - DMA x in natural layout, casting fp32->fp16 in the SWDGE DMA.
  - bn_stats/bn_aggr on DVE for per-row mean/var.
  - PE transposes x into [d, i] tiles (fp16), ACT copies PSUM->SBUF.
  - PE streams the transposed tiles against w1^T (fp16, N=512, full rate),
    accumulating y0_raw = x @ w1^T in PSUM over 24 K-chunks.
  - Normalization is folded in afterwards: (x-m)/s @ w1^T =
    (y0_raw - m * rowsum(w1)) / s.  The mean-correction is a K=1 fp32
    matmul accumulated into the same PSUM group; the 1/s scaling is a DVE
    multiply against a partition-broadcast row vector.
  - Layers 2/3 are small fp32 matmuls in the transposed layout where the
    biases are per-partition ACT Lrelu bias APs.
  - PE transposes the [10, 512] result back to natural [512, 10] and DMAs out.
"""
import os
import sys

for _p in ("/opt/trn_rl_repo", "/root/.axon_site/_ro/trn_rl_repo"):
    if os.path.isdir(_p) and _p not in sys.path:
        sys.path.append(_p)

import numpy as np

import concourse.bass as bass
import concourse.bacc as bacc
import concourse.tile as tile
from concourse import mybir
from concourse.bass_utils import run_bass_kernel_spmd

F32 = mybir.dt.float32
F16 = mybir.dt.float16
AF = mybir.ActivationFunctionType

N_CORES = 8
B = 32768
D = 3072
H = 32
O = 10
B_CORE = B // N_CORES      # 4096
IBLK = 512                 # rows per block
NSUB = IBLK // 128         # 4 sub-tiles of 128 rows
NBLK = B_CORE // IBLK      # 8
NCHUNK = D // 128          # 24 contraction chunks
DDOF_SCALE = float(D) / float(D - 1)

LAST_EXEC_NS = None
_CACHE = {}


def _build():
    nc = bacc.Bacc("TRN2", target_bir_lowering=False, debug=False, num_devices=1)

    x_d = nc.dram_tensor("x", [B_CORE, D], F32, kind="ExternalInput").ap()
    w1t_d = nc.dram_tensor("w1t", [D, H], F16, kind="ExternalInput").ap()
    w2t_d = nc.dram_tensor("w2t", [H, H], F32, kind="ExternalInput").ap()
    w3t_d = nc.dram_tensor("w3t", [H, O], F32, kind="ExternalInput").ap()
    negs_d = nc.dram_tensor("negs", [1, H], F32, kind="ExternalInput").ap()
    b1_d = nc.dram_tensor("b1c", [H, 1], F32, kind="ExternalInput").ap()
    b2_d = nc.dram_tensor("b2c", [H, 1], F32, kind="ExternalInput").ap()
    b3_d = nc.dram_tensor("b3c", [O, 1], F32, kind="ExternalInput").ap()
    idh_d = nc.dram_tensor("idh", [128, 128], F16, kind="ExternalInput").ap()
    idf_d = nc.dram_tensor("idf", [128, 128], F32, kind="ExternalInput").ap()
    y_d = nc.dram_tensor("y", [B_CORE, O], F32, kind="ExternalOutput").ap()

    with tile.TileContext(nc) as tc:
        with tc.tile_pool(name="consts", bufs=1) as consts, \
             tc.tile_pool(name="xpool", bufs=6) as xpool, \
             tc.tile_pool(name="xtpool", bufs=4) as xtpool, \
             tc.tile_pool(name="spool", bufs=3) as spool, \
             tc.tile_pool(name="opool", bufs=2) as opool, \
             tc.tile_pool(name="pxt", bufs=2, space="PSUM") as pxt_pool, \
             tc.tile_pool(name="py0", bufs=2, space="PSUM") as py0_pool, \
             tc.tile_pool(name="pst", bufs=2, space="PSUM") as pst_pool, \
             tc.tile_pool(name="pl", bufs=2, space="PSUM") as pl_pool:

            # ---- constants ----
            w1t_sb = consts.tile([128, NCHUNK, H], F16)
            nc.sync.dma_start(
                out=w1t_sb,
                in_=w1t_d.rearrange("(c p) h -> p c h", p=128),
            )
            w2t_sb = consts.tile([H, H], F32)
            nc.sync.dma_start(out=w2t_sb, in_=w2t_d)
            w3t_sb = consts.tile([H, O], F32)
            nc.sync.dma_start(out=w3t_sb, in_=w3t_d)
            negs_sb = consts.tile([1, H], F32)
            nc.sync.dma_start(out=negs_sb, in_=negs_d)
            b1_sb = consts.tile([H, 1], F32)
            nc.sync.dma_start(out=b1_sb, in_=b1_d)
            b2_sb = consts.tile([H, 1], F32)
            nc.sync.dma_start(out=b2_sb, in_=b2_d)
            b3_sb = consts.tile([O, 1], F32)
            nc.sync.dma_start(out=b3_sb, in_=b3_d)
            idh_sb = consts.tile([128, 128], F16)
            nc.sync.dma_start(out=idh_sb, in_=idh_d)
            idf_sb = consts.tile([128, 128], F32)
            nc.sync.dma_start(out=idf_sb, in_=idf_d)

            for b in range(NBLK):
                r0 = b * IBLK
                # ---- load x block (fp32 -> fp16 cast in DMA) ----
                xs = []
                for s in range(NSUB):
                    xt = xpool.tile([128, D], F16, tag="xnat")
                    nc.gpsimd.dma_start(
                        out=xt, in_=x_d[r0 + s * 128:r0 + (s + 1) * 128, :]
                    )
                    xs.append(xt)

                # ---- per-row stats on DVE ----
                mvs = []
                for s in range(NSUB):
                    st6 = spool.tile([128, 6, 6], F32, tag="st6")
                    for k in range(6):
                        nc.vector.bn_stats(
                            out=st6[:, k, :], in_=xs[s][:, k * 512:(k + 1) * 512]
                        )
                    mv = spool.tile([128, 2], F32, tag="mv")
                    nc.vector.bn_aggr(out=mv, in_=st6)
                    mvs.append(mv)

                # ---- stats to row layout: [128,1] cols -> [1, 512] psum rows ----
                pmean = pst_pool.tile([1, IBLK], F32, tag="pstat")
                pvar = pst_pool.tile([1, IBLK], F32, tag="pstat")
                for s in range(NSUB):
                    nc.tensor.transpose(
                        pmean[:, s * 128:(s + 1) * 128], mvs[s][:, 0:1], idf_sb
                    )
                    nc.tensor.transpose(
                        pvar[:, s * 128:(s + 1) * 128], mvs[s][:, 1:2], idf_sb
                    )
                mean_row = spool.tile([1, IBLK], F32, tag="mrow")
                nc.scalar.copy(mean_row, pmean)
                std_row = spool.tile([1, IBLK], F32, tag="srow")
                nc.scalar.activation(std_row, pvar, AF.Sqrt, scale=DDOF_SCALE)
                inv_row = spool.tile([1, IBLK], F32, tag="irow")
                nc.vector.reciprocal(inv_row, std_row)
                inv_b = spool.tile([H, IBLK], F32, tag="invb")
                nc.gpsimd.partition_broadcast(inv_b, inv_row)

                # ---- transpose x + stream against w1t ----
                py0 = py0_pool.tile([H, IBLK], F32)
                for c in range(NCHUNK):
                    pxt = pxt_pool.tile([128, IBLK], F16)
                    for s in range(NSUB):
                        nc.tensor.transpose(
                            pxt[:, s * 128:(s + 1) * 128],
                            xs[s][:, c * 128:(c + 1) * 128],
                            idh_sb,
                        )
                    xts = xtpool.tile([128, IBLK], F16, tag="xt")
                    nc.scalar.copy(xts, pxt)
                    nc.tensor.matmul(
                        py0, w1t_sb[:, c, :], xts, start=(c == 0), stop=False
                    )
                # mean correction: y0 -= rowsum(w1) (x) mean  (K=1 fp32 matmul)
                nc.tensor.matmul(py0, negs_sb, mean_row, start=False, stop=True)

                # ---- normalize + layer 1 activation ----
                t1 = spool.tile([H, IBLK], F32, tag="t1")
                nc.vector.tensor_mul(t1, py0, inv_b)
                h1 = spool.tile([H, IBLK], F32, tag="h1")
                nc.scalar.activation(h1, t1, AF.Lrelu, bias=b1_sb, scale=1.0,
                                     alpha=0.01)

                # ---- layers 2 and 3 (small fp32 matmuls) ----
                p2 = pl_pool.tile([H, IBLK], F32, tag="pl")
                nc.tensor.matmul(p2, w2t_sb, h1, start=True, stop=True)
                h2 = spool.tile([H, IBLK], F32, tag="h2")
                nc.scalar.activation(h2, p2, AF.Lrelu, bias=b2_sb, scale=1.0,
                                     alpha=0.01)
                p3 = pl_pool.tile([O, IBLK], F32, tag="pl")
                nc.tensor.matmul(p3, w3t_sb, h2, start=True, stop=True)
                y3 = spool.tile([O, IBLK], F32, tag="y3")
                nc.scalar.activation(y3, p3, AF.Lrelu, bias=b3_sb, scale=1.0,
                                     alpha=0.01)

                # ---- back to natural layout and store ----
                pout = pl_pool.tile([128, NSUB, O], F32, tag="pl")
                for s in range(NSUB):
                    nc.tensor.transpose(
                        pout[:, s, :],
                        y3[:, s * 128:(s + 1) * 128],
                        idf_sb[0:O, 0:O],
                    )
                out_sb = opool.tile([128, NSUB, O], F32, tag="out")
                nc.vector.tensor_copy(out_sb, pout)
                nc.sync.dma_start(
                    out=y_d[r0:r0 + IBLK, :].rearrange("(s p) c -> p s c", p=128),
                    in_=out_sb,
                )

    nc.compile()
    return nc


def _prep_inputs(x, w1, b1, w2, b2, w3, b3):
    x = np.ascontiguousarray(np.asarray(x, dtype=np.float32))
    w1 = np.asarray(w1, dtype=np.float32)
    w2 = np.asarray(w2, dtype=np.float32)
    w3 = np.asarray(w3, dtype=np.float32)
    b1 = np.asarray(b1, dtype=np.float32)
    b2 = np.asarray(b2, dtype=np.float32)
    b3 = np.asarray(b3, dtype=np.float32)

    common = {
        "w1t": np.ascontiguousarray(w1.T).astype(np.float16),
        "w2t": np.ascontiguousarray(w2.T),
        "w3t": np.ascontiguousarray(w3.T),
        "negs": np.ascontiguousarray(
            -w1.astype(np.float64).sum(axis=1, keepdims=True).T
        ).astype(np.float32),
        "b1c": np.ascontiguousarray(b1[:, None]),
        "b2c": np.ascontiguousarray(b2[:, None]),
        "b3c": np.ascontiguousarray(b3[:, None]),
        "idh": np.eye(128, dtype=np.float16),
        "idf": np.eye(128, dtype=np.float32),
    }
    in_maps = []
    for c in range(N_CORES):
        m = dict(common)
        m["x"] = x[c * B_CORE:(c + 1) * B_CORE]
        in_maps.append(m)
    return in_maps


def kernel(x, w1, b1, w2, b2, w3, b3):
    global LAST_EXEC_NS
    if "nc" not in _CACHE:
        _CACHE["nc"] = _build()
    nc = _CACHE["nc"]
    in_maps = _prep_inputs(x, w1, b1, w2, b2, w3, b3)
    trace = bool(int(os.environ.get("KERNEL_PROFILE", "0")))
    res = run_bass_kernel_spmd(nc, in_maps, core_ids=list(range(N_CORES)),
                               trace=trace)
    LAST_EXEC_NS = res.exec_time_ns
    out = np.concatenate([r["y"] for r in res.results], axis=0)
    return out.astype(np.float32)


# revision 4
# speedup vs baseline: 1.1962x; 1.1962x over previous
"""Trainium2 Bass kernel for nn_NeuralNet_19250043421419.

Row-normalize x (mean/std over D=3072, ddof=1) then a 3-layer MLP
(3072->32->32->10) with LeakyReLU(0.01) after every layer.

Strategy: pure data parallel over 8 NeuronCores (batch 32768 -> 4096/core).
Per core, per 512-row block:
  - DMA x in natural layout, casting fp32->fp16 in the SWDGE DMA.
  - bn_stats/bn_aggr on DVE for per-row mean/var.
  - PE transposes x into [d, i] tiles (fp16), ACT copies PSUM->SBUF.
  - PE streams the transposed tiles against w1^T (fp16, N=512, full rate),
    accumulating y0_raw = x @ w1^T in PSUM over 24 K-chunks.
  - Normalization is folded in afterwards: (x-m)/s @ w1^T =
    (y0_raw - m * rowsum(w1)) / s.  The mean-correction is a K=1 fp32
    matmul accumulated into the same PSUM group; the 1/s scaling is a DVE
    multiply against a partition-broadcast row vector.
  - Layers 2/3 are small fp32 matmuls in the transposed layout where the
    biases are per-partition ACT Lrelu bias APs.
  - PE transposes the [10, 512] result back to natural [512, 10] and DMAs out.
"""
import os
import sys

for _p in ("/opt/trn_rl_repo", "/root/.axon_site/_ro/trn_rl_repo"):
    if os.path.isdir(_p) and _p not in sys.path:
        sys.path.append(_p)

import numpy as np

import concourse.bass as bass
import concourse.bacc as bacc
import concourse.tile as tile
from concourse import mybir
from concourse.bass_utils import run_bass_kernel_spmd

F32 = mybir.dt.float32
F16 = mybir.dt.float16
AF = mybir.ActivationFunctionType

N_CORES = 8
B = 32768
D = 3072
H = 32
O = 10
B_CORE = B // N_CORES      # 4096
IBLK = 512                 # rows per block
NSUB = IBLK // 128         # 4 sub-tiles of 128 rows
NBLK = B_CORE // IBLK      # 8
NCHUNK = D // 128          # 24 contraction chunks
DDOF_SCALE = float(D) / float(D - 1)

LAST_EXEC_NS = None
_CACHE = {}


def _build():
    nc = bacc.Bacc("TRN2", target_bir_lowering=False, debug=False, num_devices=1)

    x_d = nc.dram_tensor("x", [B_CORE, D], F32, kind="ExternalInput").ap()
    w1t_d = nc.dram_tensor("w1t", [D, H], F16, kind="ExternalInput").ap()
    w2t_d = nc.dram_tensor("w2t", [H, H], F32, kind="ExternalInput").ap()
    w3t_d = nc.dram_tensor("w3t", [H, O], F32, kind="ExternalInput").ap()
    negs_d = nc.dram_tensor("negs", [1, H], F32, kind="ExternalInput").ap()
    b1_d = nc.dram_tensor("b1c", [H, 1], F32, kind="ExternalInput").ap()
    b2_d = nc.dram_tensor("b2c", [H, 1], F32, kind="ExternalInput").ap()
    b3_d = nc.dram_tensor("b3c", [O, 1], F32, kind="ExternalInput").ap()
    idh_d = nc.dram_tensor("idh", [128, 128], F16, kind="ExternalInput").ap()
    idf_d = nc.dram_tensor("idf", [128, 128], F32, kind="ExternalInput").ap()
    y_d = nc.dram_tensor("y", [B_CORE, O], F32, kind="ExternalOutput").ap()

    with tile.TileContext(nc) as tc:
        with tc.tile_pool(name="consts", bufs=1) as consts, \
             tc.tile_pool(name="xpool", bufs=6) as xpool, \
             tc.tile_pool(name="xtpool", bufs=3) as xtpool, \
             tc.tile_pool(name="spool", bufs=3) as spool, \
             tc.tile_pool(name="opool", bufs=2) as opool, \
             tc.tile_pool(name="pxt", bufs=2, space="PSUM") as pxt_pool, \
             tc.tile_pool(name="py0", bufs=2, space="PSUM") as py0_pool, \
             tc.tile_pool(name="pl", bufs=2, space="PSUM") as pl_pool:

            # ---- constants ----
            w1t_sb = consts.tile([128, NCHUNK, H], F16)
            nc.sync.dma_start(
                out=w1t_sb,
                in_=w1t_d.rearrange("(c p) h -> p c h", p=128),
            )
            w2t_sb = consts.tile([H, H], F32)
            nc.sync.dma_start(out=w2t_sb, in_=w2t_d)
            w3t_sb = consts.tile([H, O], F32)
            nc.sync.dma_start(out=w3t_sb, in_=w3t_d)
            negs_sb = consts.tile([1, H], F32)
            nc.sync.dma_start(out=negs_sb, in_=negs_d)
            b1_sb = consts.tile([H, 1], F32)
            nc.sync.dma_start(out=b1_sb, in_=b1_d)
            b2_sb = consts.tile([H, 1], F32)
            nc.sync.dma_start(out=b2_sb, in_=b2_d)
            b3_sb = consts.tile([O, 1], F32)
            nc.sync.dma_start(out=b3_sb, in_=b3_d)
            idh_sb = consts.tile([128, 128], F16)
            nc.sync.dma_start(out=idh_sb, in_=idh_d)
            idf_sb = consts.tile([128, 128], F32)
            nc.sync.dma_start(out=idf_sb, in_=idf_d)

            for b in range(NBLK):
                r0 = b * IBLK
                # ---- load x block (fp32 -> fp16 cast in DMA) ----
                xs = []
                for s in range(NSUB):
                    xt = xpool.tile([128, D], F16, tag="xnat")
                    nc.gpsimd.dma_start(
                        out=xt, in_=x_d[r0 + s * 128:r0 + (s + 1) * 128, :]
                    )
                    xs.append(xt)

                # ---- per-row stats on DVE; 1/std per 128-col on ACT ----
                mvs = []
                invs = []
                for s in range(NSUB):
                    st6 = spool.tile([128, 6, 6], F32, tag="st6")
                    for k in range(6):
                        nc.vector.bn_stats(
                            out=st6[:, k, :], in_=xs[s][:, k * 512:(k + 1) * 512]
                        )
                    mv = spool.tile([128, 2], F32, tag="mv")
                    nc.vector.bn_aggr(out=mv, in_=st6)
                    mvs.append(mv)
                    inv_col = spool.tile([128, 1], F32, tag="invc")
                    nc.scalar.activation(inv_col, mv[:, 1:2],
                                         AF.Abs_reciprocal_sqrt, scale=DDOF_SCALE)
                    invs.append(inv_col)

                # ---- stats to row layout: [128,1] cols -> [1, 512] psum rows ----
                pmean = pl_pool.tile([1, IBLK], F32, tag="pl")
                pinv = pl_pool.tile([1, IBLK], F32, tag="pl")
                for s in range(NSUB):
                    nc.tensor.transpose(
                        pmean[:, s * 128:(s + 1) * 128], mvs[s][:, 0:1], idf_sb
                    )
                    nc.tensor.transpose(
                        pinv[:, s * 128:(s + 1) * 128], invs[s], idf_sb
                    )
                mean_row = spool.tile([1, IBLK], F32, tag="mrow")
                nc.scalar.copy(mean_row, pmean)
                inv_row = spool.tile([1, IBLK], F32, tag="irow")
                nc.scalar.copy(inv_row, pinv)
                inv_b = spool.tile([H, IBLK], F32, tag="invb")
                nc.gpsimd.partition_broadcast(inv_b, inv_row)

                # ---- transpose x (as regular fp16 matmuls vs identity, to
                # keep the PE HAM-warm) + stream against w1t ----
                py0 = py0_pool.tile([H, IBLK], F32)
                for c2 in range(NCHUNK // 2):
                    pxt = pxt_pool.tile([128, 2 * IBLK], F32)
                    for q in range(2):
                        c = 2 * c2 + q
                        for s in range(NSUB):
                            nc.tensor.matmul(
                                pxt[:, q * IBLK + s * 128:q * IBLK + (s + 1) * 128],
                                xs[s][:, c * 128:(c + 1) * 128],
                                idh_sb,
                                start=True, stop=True,
                            )
                    xts = xtpool.tile([128, 2 * IBLK], F16, tag="xt")
                    nc.scalar.copy(xts, pxt)
                    for q in range(2):
                        c = 2 * c2 + q
                        nc.tensor.matmul(
                            py0, w1t_sb[:, c, :],
                            xts[:, q * IBLK:(q + 1) * IBLK],
                            start=(c == 0), stop=False,
                        )
                # mean correction: y0 -= rowsum(w1) (x) mean  (K=1 fp32 matmul)
                nc.tensor.matmul(py0, negs_sb, mean_row, start=False, stop=True)

                # ---- normalize + layer 1 activation ----
                t1 = spool.tile([H, IBLK], F32, tag="t1")
                nc.vector.tensor_mul(t1, py0, inv_b)
                h1 = spool.tile([H, IBLK], F32, tag="h1")
                nc.scalar.activation(h1, t1, AF.Lrelu, bias=b1_sb, scale=1.0,
                                     alpha=0.01)

                # ---- layers 2 and 3 (small fp32 matmuls) ----
                p2 = pl_pool.tile([H, IBLK], F32, tag="pl")
                nc.tensor.matmul(p2, w2t_sb, h1, start=True, stop=True)
                h2 = spool.tile([H, IBLK], F32, tag="h2")
                nc.scalar.activation(h2, p2, AF.Lrelu, bias=b2_sb, scale=1.0,
                                     alpha=0.01)
                p3 = pl_pool.tile([O, IBLK], F32, tag="pl")
                nc.tensor.matmul(p3, w3t_sb, h2, start=True, stop=True)
                y3 = spool.tile([O, IBLK], F32, tag="y3")
                nc.scalar.activation(y3, p3, AF.Lrelu, bias=b3_sb, scale=1.0,
                                     alpha=0.01)

                # ---- back to natural layout and store ----
                pout = pl_pool.tile([128, NSUB, O], F32, tag="pl")
                for s in range(NSUB):
                    nc.tensor.transpose(
                        pout[:, s, :],
                        y3[:, s * 128:(s + 1) * 128],
                        idf_sb[0:O, 0:O],
                    )
                out_sb = opool.tile([128, NSUB, O], F32, tag="out")
                nc.vector.tensor_copy(out_sb, pout)
                nc.sync.dma_start(
                    out=y_d[r0:r0 + IBLK, :].rearrange("(s p) c -> p s c", p=128),
                    in_=out_sb,
                )

    nc.compile()
    return nc


def _prep_inputs(x, w1, b1, w2, b2, w3, b3):
    x = np.ascontiguousarray(np.asarray(x, dtype=np.float32))
    w1 = np.asarray(w1, dtype=np.float32)
    w2 = np.asarray(w2, dtype=np.float32)
    w3 = np.asarray(w3, dtype=np.float32)
    b1 = np.asarray(b1, dtype=np.float32)
    b2 = np.asarray(b2, dtype=np.float32)
    b3 = np.asarray(b3, dtype=np.float32)

    common = {
        "w1t": np.ascontiguousarray(w1.T).astype(np.float16),
        "w2t": np.ascontiguousarray(w2.T),
        "w3t": np.ascontiguousarray(w3.T),
        "negs": np.ascontiguousarray(
            -w1.astype(np.float64).sum(axis=1, keepdims=True).T
        ).astype(np.float32),
        "b1c": np.ascontiguousarray(b1[:, None]),
        "b2c": np.ascontiguousarray(b2[:, None]),
        "b3c": np.ascontiguousarray(b3[:, None]),
        "idh": np.eye(128, dtype=np.float16),
        "idf": np.eye(128, dtype=np.float32),
    }
    in_maps = []
    for c in range(N_CORES):
        m = dict(common)
        m["x"] = x[c * B_CORE:(c + 1) * B_CORE]
        in_maps.append(m)
    return in_maps


def kernel(x, w1, b1, w2, b2, w3, b3):
    global LAST_EXEC_NS
    if "nc" not in _CACHE:
        _CACHE["nc"] = _build()
    nc = _CACHE["nc"]
    in_maps = _prep_inputs(x, w1, b1, w2, b2, w3, b3)
    trace = bool(int(os.environ.get("KERNEL_PROFILE", "0")))
    res = run_bass_kernel_spmd(nc, in_maps, core_ids=list(range(N_CORES)),
                               trace=trace)
    LAST_EXEC_NS = res.exec_time_ns
    out = np.concatenate([r["y"] for r in res.results], axis=0)
    return out.astype(np.float32)


# revision 5
# speedup vs baseline: 1.2493x; 1.0444x over previous
"""Trainium2 Bass kernel for nn_NeuralNet_19250043421419.

Row-normalize x (mean/std over D=3072, ddof=1) then a 3-layer MLP
(3072->32->32->10) with LeakyReLU(0.01) after every layer.

Strategy: pure data parallel over 8 NeuronCores (batch 32768 -> 4096/core).
Per core, per 512-row block:
  - DMA x in natural layout, casting fp32->fp16 in the SWDGE DMA.
  - bn_stats/bn_aggr on DVE for per-row mean/var.
  - PE transposes x into [d, i] tiles (fp16), ACT copies PSUM->SBUF.
  - PE streams the transposed tiles against w1^T (fp16, N=512, full rate),
    accumulating y0_raw = x @ w1^T in PSUM over 24 K-chunks.
  - Normalization is folded in afterwards: (x-m)/s @ w1^T =
    (y0_raw - m * rowsum(w1)) / s.  The mean-correction is a K=1 fp32
    matmul accumulated into the same PSUM group; the 1/s scaling is a DVE
    multiply against a partition-broadcast row vector.
  - Layers 2/3 are small fp32 matmuls in the transposed layout where the
    biases are per-partition ACT Lrelu bias APs.
  - PE transposes the [10, 512] result back to natural [512, 10] and DMAs out.
"""
import os
import sys

for _p in ("/opt/trn_rl_repo", "/root/.axon_site/_ro/trn_rl_repo"):
    if os.path.isdir(_p) and _p not in sys.path:
        sys.path.append(_p)

import numpy as np

import concourse.bass as bass
import concourse.bacc as bacc
import concourse.tile as tile
from concourse import mybir
from concourse.bass_utils import run_bass_kernel_spmd

F32 = mybir.dt.float32
F16 = mybir.dt.float16
AF = mybir.ActivationFunctionType

N_CORES = 8
B = 32768
D = 3072
H = 32
O = 10
B_CORE = B // N_CORES      # 4096
IBLK = 512                 # rows per block
NSUB = IBLK // 128         # 4 sub-tiles of 128 rows
NBLK = B_CORE // IBLK      # 8
NCHUNK = D // 128          # 24 contraction chunks
DDOF_SCALE = float(D) / float(D - 1)

LAST_EXEC_NS = None
_CACHE = {}


def _build():
    nc = bacc.Bacc("TRN2", target_bir_lowering=False, debug=False, num_devices=1)

    x_d = nc.dram_tensor("x", [B_CORE, D], F32, kind="ExternalInput").ap()
    w1t_d = nc.dram_tensor("w1t", [D, H], F16, kind="ExternalInput").ap()
    w2t_d = nc.dram_tensor("w2t", [H, H], F32, kind="ExternalInput").ap()
    w3t_d = nc.dram_tensor("w3t", [H, O], F32, kind="ExternalInput").ap()
    negs_d = nc.dram_tensor("negs", [1, H], F32, kind="ExternalInput").ap()
    b1_d = nc.dram_tensor("b1c", [H, 1], F32, kind="ExternalInput").ap()
    b2_d = nc.dram_tensor("b2c", [H, 1], F32, kind="ExternalInput").ap()
    b3_d = nc.dram_tensor("b3c", [O, 1], F32, kind="ExternalInput").ap()
    idh_d = nc.dram_tensor("idh", [128, 128], F16, kind="ExternalInput").ap()
    idf_d = nc.dram_tensor("idf", [128, 128], F32, kind="ExternalInput").ap()
    y_d = nc.dram_tensor("y", [B_CORE, O], F32, kind="ExternalOutput").ap()

    with tile.TileContext(nc) as tc:
        with tc.tile_pool(name="consts", bufs=1) as consts, \
             tc.tile_pool(name="xpool", bufs=6) as xpool, \
             tc.tile_pool(name="xtpool", bufs=3) as xtpool, \
             tc.tile_pool(name="spool", bufs=3) as spool, \
             tc.tile_pool(name="opool", bufs=2) as opool, \
             tc.tile_pool(name="pxt", bufs=2, space="PSUM") as pxt_pool, \
             tc.tile_pool(name="py0", bufs=2, space="PSUM") as py0_pool, \
             tc.tile_pool(name="pl", bufs=2, space="PSUM") as pl_pool:

            # ---- constants ----
            w1t_sb = consts.tile([128, NCHUNK, H], F16)
            nc.sync.dma_start(
                out=w1t_sb,
                in_=w1t_d.rearrange("(c p) h -> p c h", p=128),
            )
            w2t_sb = consts.tile([H, H], F32)
            nc.sync.dma_start(out=w2t_sb, in_=w2t_d)
            w3t_sb = consts.tile([H, O], F32)
            nc.sync.dma_start(out=w3t_sb, in_=w3t_d)
            negs_sb = consts.tile([1, H], F32)
            nc.sync.dma_start(out=negs_sb, in_=negs_d)
            b1_sb = consts.tile([H, 1], F32)
            nc.sync.dma_start(out=b1_sb, in_=b1_d)
            b2_sb = consts.tile([H, 1], F32)
            nc.sync.dma_start(out=b2_sb, in_=b2_d)
            b3_sb = consts.tile([O, 1], F32)
            nc.sync.dma_start(out=b3_sb, in_=b3_d)
            idh_sb = consts.tile([128, 128], F16)
            nc.sync.dma_start(out=idh_sb, in_=idh_d)
            idf_sb = consts.tile([128, 128], F32)
            nc.sync.dma_start(out=idf_sb, in_=idf_d)

            for b in range(NBLK):
                r0 = b * IBLK
                # ---- load x block (fp32 -> fp16 cast in DMA) ----
                xs = []
                for s in range(NSUB):
                    xt = xpool.tile([128, D], F16, tag="xnat")
                    nc.gpsimd.dma_start(
                        out=xt, in_=x_d[r0 + s * 128:r0 + (s + 1) * 128, :]
                    )
                    xs.append(xt)

                # ---- per-row stats on DVE; 1/std per 128-col on ACT ----
                mvs = []
                invs = []
                for s in range(NSUB):
                    st6 = spool.tile([128, 6, 6], F32, tag="st6")
                    for k in range(6):
                        nc.vector.bn_stats(
                            out=st6[:, k, :], in_=xs[s][:, k * 512:(k + 1) * 512]
                        )
                    mv = spool.tile([128, 2], F32, tag="mv")
                    nc.vector.bn_aggr(out=mv, in_=st6)
                    mvs.append(mv)
                    inv_col = spool.tile([128, 1], F32, tag="invc")
                    nc.scalar.activation(inv_col, mv[:, 1:2],
                                         AF.Abs_reciprocal_sqrt, scale=DDOF_SCALE)
                    invs.append(inv_col)

                # ---- stats to row layout: [128,1] cols -> [1, 512] psum rows ----
                pmean = pl_pool.tile([1, IBLK], F32, tag="pl")
                pinv = pl_pool.tile([1, IBLK], F32, tag="pl")
                for s in range(NSUB):
                    nc.tensor.transpose(
                        pmean[:, s * 128:(s + 1) * 128], mvs[s][:, 0:1], idf_sb
                    )
                    nc.tensor.transpose(
                        pinv[:, s * 128:(s + 1) * 128], invs[s], idf_sb
                    )
                mean_row = spool.tile([1, IBLK], F32, tag="mrow")
                nc.scalar.copy(mean_row, pmean)
                inv_row = spool.tile([1, IBLK], F32, tag="irow")
                nc.scalar.copy(inv_row, pinv)
                inv_b = spool.tile([H, IBLK], F32, tag="invb")
                nc.gpsimd.partition_broadcast(inv_b, inv_row)

                # ---- transpose x (as regular fp16 matmuls vs identity, to
                # keep the PE HAM-warm) + stream against w1t ----
                py0 = py0_pool.tile([H, IBLK], F32)
                prev = None
                for c2 in range(NCHUNK // 2):
                    pxt = pxt_pool.tile([128, 2 * IBLK], F32)
                    for q in range(2):
                        c = 2 * c2 + q
                        for s in range(NSUB):
                            nc.tensor.matmul(
                                pxt[:, q * IBLK + s * 128:q * IBLK + (s + 1) * 128],
                                xs[s][:, c * 128:(c + 1) * 128],
                                idh_sb,
                                start=True, stop=True,
                            )
                    xts = xtpool.tile([128, 2 * IBLK], F16, tag="xt")
                    nc.scalar.copy(xts, pxt)
                    if prev is not None:
                        pc2, pxts = prev
                        for q in range(2):
                            c = 2 * pc2 + q
                            nc.tensor.matmul(
                                py0, w1t_sb[:, c, :],
                                pxts[:, q * IBLK:(q + 1) * IBLK],
                                start=(c == 0), stop=False,
                            )
                    prev = (c2, xts)
                pc2, pxts = prev
                for q in range(2):
                    c = 2 * pc2 + q
                    nc.tensor.matmul(
                        py0, w1t_sb[:, c, :],
                        pxts[:, q * IBLK:(q + 1) * IBLK],
                        start=False, stop=False,
                    )
                # mean correction: y0 -= rowsum(w1) (x) mean  (K=1 fp32 matmul)
                nc.tensor.matmul(py0, negs_sb, mean_row, start=False, stop=True)

                # ---- normalize + layer 1 activation ----
                t1 = spool.tile([H, IBLK], F32, tag="t1")
                nc.vector.tensor_mul(t1, py0, inv_b)
                h1 = spool.tile([H, IBLK], F32, tag="h1")
                nc.scalar.activation(h1, t1, AF.Prelu, bias=b1_sb, scale=1.0,
                                     alpha=0.01)

                # ---- layers 2 and 3 (small fp32 matmuls) ----
                p2 = pl_pool.tile([H, IBLK], F32, tag="pl")
                nc.tensor.matmul(p2, w2t_sb, h1, start=True, stop=True)
                h2 = spool.tile([H, IBLK], F32, tag="h2")
                nc.scalar.activation(h2, p2, AF.Prelu, bias=b2_sb, scale=1.0,
                                     alpha=0.01)
                p3 = pl_pool.tile([O, IBLK], F32, tag="pl")
                nc.tensor.matmul(p3, w3t_sb, h2, start=True, stop=True)
                y3 = spool.tile([O, IBLK], F32, tag="y3")
                nc.scalar.activation(y3, p3, AF.Prelu, bias=b3_sb, scale=1.0,
                                     alpha=0.01)

                # ---- back to natural layout and store ----
                pout = pl_pool.tile([128, NSUB, O], F32, tag="pl")
                for s in range(NSUB):
                    nc.tensor.transpose(
                        pout[:, s, :],
                        y3[:, s * 128:(s + 1) * 128],
                        idf_sb[0:O, 0:O],
                    )
                out_sb = opool.tile([128, NSUB, O], F32, tag="out")
                nc.vector.tensor_copy(out_sb, pout)
                nc.sync.dma_start(
                    out=y_d[r0:r0 + IBLK, :].rearrange("(s p) c -> p s c", p=128),
                    in_=out_sb,
                )

    nc.compile()
    return nc


def _prep_inputs(x, w1, b1, w2, b2, w3, b3):
    x = np.ascontiguousarray(np.asarray(x, dtype=np.float32))
    w1 = np.asarray(w1, dtype=np.float32)
    w2 = np.asarray(w2, dtype=np.float32)
    w3 = np.asarray(w3, dtype=np.float32)
    b1 = np.asarray(b1, dtype=np.float32)
    b2 = np.asarray(b2, dtype=np.float32)
    b3 = np.asarray(b3, dtype=np.float32)

    common = {
        "w1t": np.ascontiguousarray(w1.T).astype(np.float16),
        "w2t": np.ascontiguousarray(w2.T),
        "w3t": np.ascontiguousarray(w3.T),
        "negs": np.ascontiguousarray(
            -w1.astype(np.float64).sum(axis=1, keepdims=True).T
        ).astype(np.float32),
        "b1c": np.ascontiguousarray(b1[:, None]),
        "b2c": np.ascontiguousarray(b2[:, None]),
        "b3c": np.ascontiguousarray(b3[:, None]),
        "idh": np.eye(128, dtype=np.float16),
        "idf": np.eye(128, dtype=np.float32),
    }
    in_maps = []
    for c in range(N_CORES):
        m = dict(common)
        m["x"] = x[c * B_CORE:(c + 1) * B_CORE]
        in_maps.append(m)
    return in_maps


def kernel(x, w1, b1, w2, b2, w3, b3):
    global LAST_EXEC_NS
    if "nc" not in _CACHE:
        _CACHE["nc"] = _build()
    nc = _CACHE["nc"]
    in_maps = _prep_inputs(x, w1, b1, w2, b2, w3, b3)
    trace = bool(int(os.environ.get("KERNEL_PROFILE", "0")))
    res = run_bass_kernel_spmd(nc, in_maps, core_ids=list(range(N_CORES)),
                               trace=trace)
    LAST_EXEC_NS = res.exec_time_ns
    out = np.concatenate([r["y"] for r in res.results], axis=0)
    return out.astype(np.float32)


# revision 6
# speedup vs baseline: 1.5709x; 1.2574x over previous
"""Trainium2 Bass kernel for nn_NeuralNet_19250043421419.

Row-normalize x (mean/std over D=3072, ddof=1) then a 3-layer MLP
(3072->32->32->10) with LeakyReLU(0.01) after every layer.

Strategy: pure data parallel over 8 NeuronCores (batch 32768 -> 4096/core).
Per core, per 512-row block:
  - DMA x in natural layout, casting fp32->fp16 in the SWDGE DMA.
  - bn_stats/bn_aggr on DVE for per-row mean/var.
  - PE transposes x into [d, i] tiles (fp16), ACT copies PSUM->SBUF.
  - PE streams the transposed tiles against w1^T (fp16, N=512, full rate),
    accumulating y0_raw = x @ w1^T in PSUM over 24 K-chunks.
  - Normalization is folded in afterwards: (x-m)/s @ w1^T =
    (y0_raw - m * rowsum(w1)) / s.  The mean-correction is a K=1 fp32
    matmul accumulated into the same PSUM group; the 1/s scaling is a DVE
    multiply against a partition-broadcast row vector.
  - Layers 2/3 are small fp32 matmuls in the transposed layout where the
    biases are per-partition ACT Lrelu bias APs.
  - PE transposes the [10, 512] result back to natural [512, 10] and DMAs out.
"""
import os
import sys

for _p in ("/opt/trn_rl_repo", "/root/.axon_site/_ro/trn_rl_repo"):
    if os.path.isdir(_p) and _p not in sys.path:
        sys.path.append(_p)

import numpy as np

import concourse.bass as bass
import concourse.bacc as bacc
import concourse.tile as tile
from concourse import mybir
from concourse.bass_utils import run_bass_kernel_spmd

F32 = mybir.dt.float32
F16 = mybir.dt.float16
AF = mybir.ActivationFunctionType

N_CORES = 8
B = 32768
D = 3072
H = 32
O = 10
B_CORE = B // N_CORES      # 4096
IBLK = 512                 # rows per block
NSUB = IBLK // 128         # 4 sub-tiles of 128 rows
NBLK = B_CORE // IBLK      # 8
NCHUNK = D // 128          # 24 contraction chunks
DDOF_SCALE = float(D) / float(D - 1)

LAST_EXEC_NS = None
_CACHE = {}


def _build():
    nc = bacc.Bacc("TRN2", target_bir_lowering=False, debug=False, num_devices=1)

    x_d = nc.dram_tensor("x", [B_CORE, D], F32, kind="ExternalInput").ap()
    w1t_d = nc.dram_tensor("w1t", [D, H], F16, kind="ExternalInput").ap()
    w2t_d = nc.dram_tensor("w2t", [H, H], F32, kind="ExternalInput").ap()
    w3t_d = nc.dram_tensor("w3t", [H, O], F32, kind="ExternalInput").ap()
    negs_d = nc.dram_tensor("negs", [1, H], F32, kind="ExternalInput").ap()
    b1_d = nc.dram_tensor("b1c", [H, 1], F32, kind="ExternalInput").ap()
    b2_d = nc.dram_tensor("b2c", [H, 1], F32, kind="ExternalInput").ap()
    b3_d = nc.dram_tensor("b3c", [O, 1], F32, kind="ExternalInput").ap()
    idh_d = nc.dram_tensor("idh", [128, 128], F16, kind="ExternalInput").ap()
    idf_d = nc.dram_tensor("idf", [128, 128], F32, kind="ExternalInput").ap()
    y_d = nc.dram_tensor("y", [B_CORE, O], F32, kind="ExternalOutput").ap()

    with tile.TileContext(nc) as tc:
        with tc.tile_pool(name="consts", bufs=1) as consts, \
             tc.tile_pool(name="xpool", bufs=12) as xpool, \
             tc.tile_pool(name="xtpool", bufs=4) as xtpool, \
             tc.tile_pool(name="spool", bufs=3) as spool, \
             tc.tile_pool(name="opool", bufs=2) as opool, \
             tc.tile_pool(name="pxt", bufs=2, space="PSUM") as pxt_pool, \
             tc.tile_pool(name="py0", bufs=2, space="PSUM") as py0_pool, \
             tc.tile_pool(name="pl", bufs=2, space="PSUM") as pl_pool:

            # ---- constants ----
            w1t_sb = consts.tile([128, NCHUNK, H], F16)
            nc.sync.dma_start(
                out=w1t_sb,
                in_=w1t_d.rearrange("(c p) h -> p c h", p=128),
            )
            w2t_sb = consts.tile([H, H], F32)
            nc.sync.dma_start(out=w2t_sb, in_=w2t_d)
            w3t_sb = consts.tile([H, O], F32)
            nc.sync.dma_start(out=w3t_sb, in_=w3t_d)
            negs_sb = consts.tile([1, H], F32)
            nc.sync.dma_start(out=negs_sb, in_=negs_d)
            b1_sb = consts.tile([H, 1], F32)
            nc.sync.dma_start(out=b1_sb, in_=b1_d)
            b2_sb = consts.tile([H, 1], F32)
            nc.sync.dma_start(out=b2_sb, in_=b2_d)
            b3_sb = consts.tile([O, 1], F32)
            nc.sync.dma_start(out=b3_sb, in_=b3_d)
            idh_sb = consts.tile([128, 128], F16)
            nc.sync.dma_start(out=idh_sb, in_=idh_d)
            idf_sb = consts.tile([128, 128], F32)
            nc.sync.dma_start(out=idf_sb, in_=idf_d)

            for b in range(NBLK):
                r0 = b * IBLK
                # ---- load x block (fp32 -> fp16 cast in DMA) ----
                xs = []
                for s in range(NSUB):
                    xt = xpool.tile([128, D], F16, tag="xnat")
                    nc.gpsimd.dma_start(
                        out=xt, in_=x_d[r0 + s * 128:r0 + (s + 1) * 128, :]
                    )
                    xs.append(xt)

                # ---- per-row stats on DVE; 1/std per 128-col on ACT ----
                mvs = []
                invs = []
                for s in range(NSUB):
                    st6 = spool.tile([128, 6, 6], F32, tag="st6")
                    for k in range(6):
                        nc.vector.bn_stats(
                            out=st6[:, k, :], in_=xs[s][:, k * 512:(k + 1) * 512]
                        )
                    mv = spool.tile([128, 2], F32, tag="mv")
                    nc.vector.bn_aggr(out=mv, in_=st6)
                    mvs.append(mv)
                    inv_col = spool.tile([128, 1], F32, tag="invc")
                    nc.scalar.activation(inv_col, mv[:, 1:2],
                                         AF.Abs_reciprocal_sqrt, scale=DDOF_SCALE)
                    invs.append(inv_col)

                # ---- stats to row layout: [128,1] cols -> [1, 512] psum rows ----
                pmean = pl_pool.tile([1, IBLK], F32, tag="pl")
                pinv = pl_pool.tile([1, IBLK], F32, tag="pl")
                for s in range(NSUB):
                    nc.tensor.transpose(
                        pmean[:, s * 128:(s + 1) * 128], mvs[s][:, 0:1], idf_sb
                    )
                    nc.tensor.transpose(
                        pinv[:, s * 128:(s + 1) * 128], invs[s], idf_sb
                    )
                mean_row = spool.tile([1, IBLK], F32, tag="mrow")
                nc.scalar.copy(mean_row, pmean)
                inv_row = spool.tile([1, IBLK], F32, tag="irow")
                nc.scalar.copy(inv_row, pinv)
                inv_b = spool.tile([H, IBLK], F32, tag="invb")
                nc.gpsimd.partition_broadcast(inv_b, inv_row)

                # ---- transpose x (as regular fp16 matmuls vs identity, to
                # keep the PE HAM-warm) + stream against w1t ----
                py0 = py0_pool.tile([H, IBLK], F32)
                prev = None
                for c2 in range(NCHUNK // 2):
                    pxt = pxt_pool.tile([128, 2 * IBLK], F32)
                    for q in range(2):
                        c = 2 * c2 + q
                        for s in range(NSUB):
                            nc.tensor.matmul(
                                pxt[:, q * IBLK + s * 128:q * IBLK + (s + 1) * 128],
                                xs[s][:, c * 128:(c + 1) * 128],
                                idh_sb,
                                start=True, stop=True,
                            )
                    xts = xtpool.tile([128, 2 * IBLK], F16, tag="xt")
                    nc.scalar.copy(xts, pxt)
                    if prev is not None:
                        pc2, pxts = prev
                        for q in range(2):
                            c = 2 * pc2 + q
                            nc.tensor.matmul(
                                py0, w1t_sb[:, c, :],
                                pxts[:, q * IBLK:(q + 1) * IBLK],
                                start=(c == 0), stop=False,
                            )
                    prev = (c2, xts)
                pc2, pxts = prev
                for q in range(2):
                    c = 2 * pc2 + q
                    nc.tensor.matmul(
                        py0, w1t_sb[:, c, :],
                        pxts[:, q * IBLK:(q + 1) * IBLK],
                        start=False, stop=False,
                    )
                # mean correction: y0 -= rowsum(w1) (x) mean  (K=1 fp32 matmul)
                nc.tensor.matmul(py0, negs_sb, mean_row, start=False, stop=True)

                # ---- normalize + layer 1 activation ----
                t1 = spool.tile([H, IBLK], F32, tag="t1")
                nc.vector.tensor_mul(t1, py0, inv_b)
                h1 = spool.tile([H, IBLK], F32, tag="h1")
                nc.scalar.activation(h1, t1, AF.Prelu, bias=b1_sb, scale=1.0,
                                     alpha=0.01)

                # ---- layers 2 and 3 (small fp32 matmuls) ----
                p2 = pl_pool.tile([H, IBLK], F32, tag="pl")
                nc.tensor.matmul(p2, w2t_sb, h1, start=True, stop=True)
                h2 = spool.tile([H, IBLK], F32, tag="h2")
                nc.scalar.activation(h2, p2, AF.Prelu, bias=b2_sb, scale=1.0,
                                     alpha=0.01)
                p3 = pl_pool.tile([O, IBLK], F32, tag="pl")
                nc.tensor.matmul(p3, w3t_sb, h2, start=True, stop=True)
                y3 = spool.tile([O, IBLK], F32, tag="y3")
                nc.scalar.activation(y3, p3, AF.Prelu, bias=b3_sb, scale=1.0,
                                     alpha=0.01)

                # ---- back to natural layout and store ----
                pout = pl_pool.tile([128, NSUB, O], F32, tag="pl")
                for s in range(NSUB):
                    nc.tensor.transpose(
                        pout[:, s, :],
                        y3[:, s * 128:(s + 1) * 128],
                        idf_sb[0:O, 0:O],
                    )
                out_sb = opool.tile([128, NSUB, O], F32, tag="out")
                nc.vector.tensor_copy(out_sb, pout)
                nc.sync.dma_start(
                    out=y_d[r0:r0 + IBLK, :].rearrange("(s p) c -> p s c", p=128),
                    in_=out_sb,
                )

    nc.compile()
    return nc


def _prep_inputs(x, w1, b1, w2, b2, w3, b3):
    x = np.ascontiguousarray(np.asarray(x, dtype=np.float32))
    w1 = np.asarray(w1, dtype=np.float32)
    w2 = np.asarray(w2, dtype=np.float32)
    w3 = np.asarray(w3, dtype=np.float32)
    b1 = np.asarray(b1, dtype=np.float32)
    b2 = np.asarray(b2, dtype=np.float32)
    b3 = np.asarray(b3, dtype=np.float32)

    common = {
        "w1t": np.ascontiguousarray(w1.T).astype(np.float16),
        "w2t": np.ascontiguousarray(w2.T),
        "w3t": np.ascontiguousarray(w3.T),
        "negs": np.ascontiguousarray(
            -w1.astype(np.float64).sum(axis=1, keepdims=True).T
        ).astype(np.float32),
        "b1c": np.ascontiguousarray(b1[:, None]),
        "b2c": np.ascontiguousarray(b2[:, None]),
        "b3c": np.ascontiguousarray(b3[:, None]),
        "idh": np.eye(128, dtype=np.float16),
        "idf": np.eye(128, dtype=np.float32),
    }
    in_maps = []
    for c in range(N_CORES):
        m = dict(common)
        m["x"] = x[c * B_CORE:(c + 1) * B_CORE]
        in_maps.append(m)
    return in_maps


def kernel(x, w1, b1, w2, b2, w3, b3):
    global LAST_EXEC_NS
    if "nc" not in _CACHE:
        _CACHE["nc"] = _build()
    nc = _CACHE["nc"]
    in_maps = _prep_inputs(x, w1, b1, w2, b2, w3, b3)
    trace = bool(int(os.environ.get("KERNEL_PROFILE", "0")))
    res = run_bass_kernel_spmd(nc, in_maps, core_ids=list(range(N_CORES)),
                               trace=trace)
    LAST_EXEC_NS = res.exec_time_ns
    out = np.concatenate([r["y"] for r in res.results], axis=0)
    return out.astype(np.float32)


# revision 7
# speedup vs baseline: 1.7878x; 1.1381x over previous
"""Trainium2 Bass kernel for nn_NeuralNet_19250043421419.

Row-normalize x (mean/std over D=3072, ddof=1) then a 3-layer MLP
(3072->32->32->10) with LeakyReLU(0.01) after every layer.

Strategy: pure data parallel over 8 NeuronCores (batch 32768 -> 4096/core).
Per core, per 512-row block:
  - DMA x in natural layout, casting fp32->fp16 in the SWDGE DMA.
  - bn_stats/bn_aggr on DVE for per-row mean/var.
  - PE transposes x into [d, i] tiles (fp16), ACT copies PSUM->SBUF.
  - PE streams the transposed tiles against w1^T (fp16, N=512, full rate),
    accumulating y0_raw = x @ w1^T in PSUM over 24 K-chunks.
  - Normalization is folded in afterwards: (x-m)/s @ w1^T =
    (y0_raw - m * rowsum(w1)) / s.  The mean-correction is a K=1 fp32
    matmul accumulated into the same PSUM group; the 1/s scaling is a DVE
    multiply against a partition-broadcast row vector.
  - Layers 2/3 are small fp32 matmuls in the transposed layout where the
    biases are per-partition ACT Lrelu bias APs.
  - PE transposes the [10, 512] result back to natural [512, 10] and DMAs out.
"""
import os
import sys

for _p in ("/opt/trn_rl_repo", "/root/.axon_site/_ro/trn_rl_repo"):
    if os.path.isdir(_p) and _p not in sys.path:
        sys.path.append(_p)

import numpy as np

import concourse.bass as bass
import concourse.bacc as bacc
import concourse.tile as tile
from concourse import mybir
from concourse.bass_utils import run_bass_kernel_spmd

F32 = mybir.dt.float32
F16 = mybir.dt.float16
AF = mybir.ActivationFunctionType

N_CORES = 8
B = 32768
D = 3072
H = 32
O = 10
B_CORE = B // N_CORES      # 4096
IBLK = 512                 # rows per block
NSUB = IBLK // 128         # 4 sub-tiles of 128 rows
NBLK = B_CORE // IBLK      # 8
NCHUNK = D // 128          # 24 contraction chunks
DDOF_SCALE = float(D) / float(D - 1)

LAST_EXEC_NS = None
_CACHE = {}


def _build():
    nc = bacc.Bacc("TRN2", target_bir_lowering=False, debug=False, num_devices=1)

    x_d = nc.dram_tensor("x", [B_CORE, D], F32, kind="ExternalInput").ap()
    w1t_d = nc.dram_tensor("w1t", [128, NCHUNK * H], F16, kind="ExternalInput").ap()
    w2t_d = nc.dram_tensor("w2t", [H, H], F32, kind="ExternalInput").ap()
    w3t_d = nc.dram_tensor("w3t", [H, O], F32, kind="ExternalInput").ap()
    negs_d = nc.dram_tensor("negs", [1, H], F32, kind="ExternalInput").ap()
    b1_d = nc.dram_tensor("b1c", [H, 1], F32, kind="ExternalInput").ap()
    b2_d = nc.dram_tensor("b2c", [H, 1], F32, kind="ExternalInput").ap()
    b3_d = nc.dram_tensor("b3c", [O, 1], F32, kind="ExternalInput").ap()
    idh_d = nc.dram_tensor("idh", [128, 128], F16, kind="ExternalInput").ap()
    idf_d = nc.dram_tensor("idf", [128, 128], F32, kind="ExternalInput").ap()
    y_d = nc.dram_tensor("y", [B_CORE, O], F32, kind="ExternalOutput").ap()

    with tile.TileContext(nc) as tc:
        with tc.tile_pool(name="consts", bufs=1) as consts, \
             tc.tile_pool(name="xpool", bufs=12) as xpool, \
             tc.tile_pool(name="xtpool", bufs=4) as xtpool, \
             tc.tile_pool(name="spool", bufs=3) as spool, \
             tc.tile_pool(name="opool", bufs=2) as opool, \
             tc.tile_pool(name="pxt", bufs=2, space="PSUM") as pxt_pool, \
             tc.tile_pool(name="py0", bufs=2, space="PSUM") as py0_pool, \
             tc.tile_pool(name="pl", bufs=2, space="PSUM") as pl_pool:

            # ---- constants ----
            w1t_sb = consts.tile([128, NCHUNK, H], F16)
            nc.sync.dma_start(
                out=w1t_sb, in_=w1t_d.rearrange("p (c h) -> p c h", h=H)
            )
            w2t_sb = consts.tile([H, H], F32)
            nc.sync.dma_start(out=w2t_sb, in_=w2t_d)
            w3t_sb = consts.tile([H, O], F32)
            nc.sync.dma_start(out=w3t_sb, in_=w3t_d)
            negs_sb = consts.tile([1, H], F32)
            nc.sync.dma_start(out=negs_sb, in_=negs_d)
            b1_sb = consts.tile([H, 1], F32)
            nc.sync.dma_start(out=b1_sb, in_=b1_d)
            b2_sb = consts.tile([H, 1], F32)
            nc.sync.dma_start(out=b2_sb, in_=b2_d)
            b3_sb = consts.tile([O, 1], F32)
            nc.sync.dma_start(out=b3_sb, in_=b3_d)
            idh_sb = consts.tile([128, 128], F16)
            nc.sync.dma_start(out=idh_sb, in_=idh_d)
            idf_sb = consts.tile([128, 128], F32)
            nc.sync.dma_start(out=idf_sb, in_=idf_d)

            for b in range(NBLK):
                r0 = b * IBLK
                # ---- load x block (fp32 -> fp16 cast in DMA) ----
                xs = []
                for s in range(NSUB):
                    xt = xpool.tile([128, D], F16, tag="xnat")
                    nc.gpsimd.dma_start(
                        out=xt, in_=x_d[r0 + s * 128:r0 + (s + 1) * 128, :]
                    )
                    xs.append(xt)

                # ---- per-row stats on DVE; 1/std per 128-col on ACT ----
                mvs = []
                invs = []
                for s in range(NSUB):
                    st6 = spool.tile([128, 6, 6], F32, tag="st6")
                    for k in range(6):
                        nc.vector.bn_stats(
                            out=st6[:, k, :], in_=xs[s][:, k * 512:(k + 1) * 512]
                        )
                    mv = spool.tile([128, 2], F32, tag="mv")
                    nc.vector.bn_aggr(out=mv, in_=st6)
                    mvs.append(mv)
                    inv_col = spool.tile([128, 1], F32, tag="invc")
                    nc.scalar.activation(inv_col, mv[:, 1:2],
                                         AF.Abs_reciprocal_sqrt, scale=DDOF_SCALE)
                    invs.append(inv_col)

                # ---- stats to row layout: [128,1] cols -> [1, 512] psum rows ----
                pmean = pl_pool.tile([1, IBLK], F32, tag="pl")
                pinv = pl_pool.tile([1, IBLK], F32, tag="pl")
                for s in range(NSUB):
                    nc.tensor.transpose(
                        pmean[:, s * 128:(s + 1) * 128], mvs[s][:, 0:1], idf_sb
                    )
                    nc.tensor.transpose(
                        pinv[:, s * 128:(s + 1) * 128], invs[s], idf_sb
                    )
                mean_row = spool.tile([1, IBLK], F32, tag="mrow")
                nc.scalar.copy(mean_row, pmean)
                inv_row = spool.tile([1, IBLK], F32, tag="irow")
                nc.scalar.copy(inv_row, pinv)
                inv_b = spool.tile([H, IBLK], F32, tag="invb")
                nc.gpsimd.partition_broadcast(inv_b, inv_row)

                # ---- transpose x (as regular fp16 matmuls vs identity, to
                # keep the PE HAM-warm) + stream against w1t ----
                py0 = py0_pool.tile([H, IBLK], F32)
                prev = None
                for c2 in range(NCHUNK // 2):
                    pxt = pxt_pool.tile([128, 2 * IBLK], F32)
                    for q in range(2):
                        c = 2 * c2 + q
                        for s in range(NSUB):
                            nc.tensor.matmul(
                                pxt[:, q * IBLK + s * 128:q * IBLK + (s + 1) * 128],
                                xs[s][:, c * 128:(c + 1) * 128],
                                idh_sb,
                                start=True, stop=True,
                            )
                    xts = xtpool.tile([128, 2 * IBLK], F16, tag="xt")
                    nc.scalar.copy(xts, pxt)
                    if prev is not None:
                        pc2, pxts = prev
                        for q in range(2):
                            c = 2 * pc2 + q
                            nc.tensor.matmul(
                                py0, w1t_sb[:, c, :],
                                pxts[:, q * IBLK:(q + 1) * IBLK],
                                start=(c == 0), stop=False,
                            )
                    prev = (c2, xts)
                pc2, pxts = prev
                for q in range(2):
                    c = 2 * pc2 + q
                    nc.tensor.matmul(
                        py0, w1t_sb[:, c, :],
                        pxts[:, q * IBLK:(q + 1) * IBLK],
                        start=False, stop=False,
                    )
                # mean correction: y0 -= rowsum(w1) (x) mean  (K=1 fp32 matmul)
                nc.tensor.matmul(py0, negs_sb, mean_row, start=False, stop=True)

                # ---- normalize + layer 1 activation ----
                t1 = spool.tile([H, IBLK], F32, tag="t1")
                nc.vector.tensor_mul(t1, py0, inv_b)
                h1 = spool.tile([H, IBLK], F32, tag="h1")
                nc.scalar.activation(h1, t1, AF.Prelu, bias=b1_sb, scale=1.0,
                                     alpha=0.01)

                # ---- layers 2 and 3 (small fp32 matmuls) ----
                p2 = pl_pool.tile([H, IBLK], F32, tag="pl")
                nc.tensor.matmul(p2, w2t_sb, h1, start=True, stop=True)
                h2 = spool.tile([H, IBLK], F32, tag="h2")
                nc.scalar.activation(h2, p2, AF.Prelu, bias=b2_sb, scale=1.0,
                                     alpha=0.01)
                p3 = pl_pool.tile([O, IBLK], F32, tag="pl")
                nc.tensor.matmul(p3, w3t_sb, h2, start=True, stop=True)
                y3 = spool.tile([O, IBLK], F32, tag="y3")
                nc.scalar.activation(y3, p3, AF.Prelu, bias=b3_sb, scale=1.0,
                                     alpha=0.01)

                # ---- back to natural layout and store ----
                pout = pl_pool.tile([128, NSUB, O], F32, tag="pl")
                for s in range(NSUB):
                    nc.tensor.transpose(
                        pout[:, s, :],
                        y3[:, s * 128:(s + 1) * 128],
                        idf_sb[0:O, 0:O],
                    )
                out_sb = opool.tile([128, NSUB, O], F32, tag="out")
                nc.vector.tensor_copy(out_sb, pout)
                nc.sync.dma_start(
                    out=y_d[r0:r0 + IBLK, :].rearrange("(s p) c -> p s c", p=128),
                    in_=out_sb,
                )

    nc.compile()
    return nc


def _prep_inputs(x, w1, b1, w2, b2, w3, b3):
    x = np.ascontiguousarray(np.asarray(x, dtype=np.float32))
    w1 = np.asarray(w1, dtype=np.float32)
    w2 = np.asarray(w2, dtype=np.float32)
    w3 = np.asarray(w3, dtype=np.float32)
    b1 = np.asarray(b1, dtype=np.float32)
    b2 = np.asarray(b2, dtype=np.float32)
    b3 = np.asarray(b3, dtype=np.float32)

    common = {
        # [128, 24*32]: partition p holds w1.T[c*128+p, :] for each chunk c
        "w1t": np.ascontiguousarray(
            w1.T.reshape(NCHUNK, 128, H).transpose(1, 0, 2).reshape(128, NCHUNK * H)
        ).astype(np.float16),
        "w2t": np.ascontiguousarray(w2.T),
        "w3t": np.ascontiguousarray(w3.T),
        "negs": np.ascontiguousarray(
            -w1.astype(np.float64).sum(axis=1, keepdims=True).T
        ).astype(np.float32),
        "b1c": np.ascontiguousarray(b1[:, None]),
        "b2c": np.ascontiguousarray(b2[:, None]),
        "b3c": np.ascontiguousarray(b3[:, None]),
        "idh": np.eye(128, dtype=np.float16),
        "idf": np.eye(128, dtype=np.float32),
    }
    in_maps = []
    for c in range(N_CORES):
        m = dict(common)
        m["x"] = x[c * B_CORE:(c + 1) * B_CORE]
        in_maps.append(m)
    return in_maps


def kernel(x, w1, b1, w2, b2, w3, b3):
    global LAST_EXEC_NS
    if "nc" not in _CACHE:
        _CACHE["nc"] = _build()
    nc = _CACHE["nc"]
    in_maps = _prep_inputs(x, w1, b1, w2, b2, w3, b3)
    trace = bool(int(os.environ.get("KERNEL_PROFILE", "0")))
    res = run_bass_kernel_spmd(nc, in_maps, core_ids=list(range(N_CORES)),
                               trace=trace)
    LAST_EXEC_NS = res.exec_time_ns
    out = np.concatenate([r["y"] for r in res.results], axis=0)
    return out.astype(np.float32)


# revision 8
# speedup vs baseline: 1.8964x; 1.0607x over previous
"""Trainium2 Bass kernel for nn_NeuralNet_19250043421419.

Row-normalize x (mean/std over D=3072, ddof=1) then a 3-layer MLP
(3072->32->32->10) with LeakyReLU(0.01) after every layer.

Strategy: pure data parallel over 8 NeuronCores (batch 32768 -> 4096/core).
Per core, per 512-row block:
  - DMA x in natural layout, casting fp32->fp16 in the SWDGE DMA.
  - bn_stats/bn_aggr on DVE for per-row mean/var.
  - PE transposes x into [d, i] tiles (fp16), ACT copies PSUM->SBUF.
  - PE streams the transposed tiles against w1^T (fp16, N=512, full rate),
    accumulating y0_raw = x @ w1^T in PSUM over 24 K-chunks.
  - Normalization is folded in afterwards: (x-m)/s @ w1^T =
    (y0_raw - m * rowsum(w1)) / s.  The mean-correction is a K=1 fp32
    matmul accumulated into the same PSUM group; the 1/s scaling is a DVE
    multiply against a partition-broadcast row vector.
  - Layers 2/3 are small fp32 matmuls in the transposed layout where the
    biases are per-partition ACT Lrelu bias APs.
  - PE transposes the [10, 512] result back to natural [512, 10] and DMAs out.
"""
import os
import sys

for _p in ("/opt/trn_rl_repo", "/root/.axon_site/_ro/trn_rl_repo"):
    if os.path.isdir(_p) and _p not in sys.path:
        sys.path.append(_p)

import numpy as np

import concourse.bass as bass
import concourse.bacc as bacc
import concourse.tile as tile
from concourse import mybir
from concourse.bass_utils import run_bass_kernel_spmd

F32 = mybir.dt.float32
F16 = mybir.dt.float16
AF = mybir.ActivationFunctionType

N_CORES = 8
B = 32768
D = 3072
H = 32
O = 10
B_CORE = B // N_CORES      # 4096
IBLK = 512                 # rows per block
NSUB = IBLK // 128         # 4 sub-tiles of 128 rows
NBLK = B_CORE // IBLK      # 8
NCHUNK = D // 128          # 24 contraction chunks
DDOF_SCALE = float(D) / float(D - 1)

LAST_EXEC_NS = None
_CACHE = {}


def _build():
    nc = bacc.Bacc("TRN2", target_bir_lowering=False, debug=False, num_devices=1)

    x_d = nc.dram_tensor("x", [B_CORE, D], F32, kind="ExternalInput").ap()
    w1t_d = nc.dram_tensor("w1t", [128, NCHUNK * H], F16, kind="ExternalInput").ap()
    w2t_d = nc.dram_tensor("w2t", [H, H], F16, kind="ExternalInput").ap()
    w3t_d = nc.dram_tensor("w3t", [H, O], F16, kind="ExternalInput").ap()
    negs_d = nc.dram_tensor("negs", [1, H], F16, kind="ExternalInput").ap()
    b1_d = nc.dram_tensor("b1c", [H, 1], F32, kind="ExternalInput").ap()
    b2_d = nc.dram_tensor("b2c", [H, 1], F32, kind="ExternalInput").ap()
    b3_d = nc.dram_tensor("b3c", [O, 1], F32, kind="ExternalInput").ap()
    idh_d = nc.dram_tensor("idh", [128, 128], F16, kind="ExternalInput").ap()
    idf_d = nc.dram_tensor("idf", [128, 128], F32, kind="ExternalInput").ap()
    y_d = nc.dram_tensor("y", [B_CORE, O], F32, kind="ExternalOutput").ap()

    with tile.TileContext(nc) as tc:
        with tc.tile_pool(name="consts", bufs=1) as consts, \
             tc.tile_pool(name="xpool", bufs=12) as xpool, \
             tc.tile_pool(name="xtpool", bufs=4) as xtpool, \
             tc.tile_pool(name="spool", bufs=3) as spool, \
             tc.tile_pool(name="opool", bufs=2) as opool, \
             tc.tile_pool(name="pxt", bufs=2, space="PSUM") as pxt_pool, \
             tc.tile_pool(name="py0", bufs=2, space="PSUM") as py0_pool, \
             tc.tile_pool(name="pl", bufs=2, space="PSUM") as pl_pool:

            # ---- constants ----
            w1t_sb = consts.tile([128, NCHUNK, H], F16)
            nc.sync.dma_start(
                out=w1t_sb, in_=w1t_d.rearrange("p (c h) -> p c h", h=H)
            )
            w2t_sb = consts.tile([H, H], F16)
            nc.sync.dma_start(out=w2t_sb, in_=w2t_d)
            w3t_sb = consts.tile([H, O], F16)
            nc.sync.dma_start(out=w3t_sb, in_=w3t_d)
            negs_sb = consts.tile([1, H], F16)
            nc.sync.dma_start(out=negs_sb, in_=negs_d)
            b1_sb = consts.tile([H, 1], F32)
            nc.sync.dma_start(out=b1_sb, in_=b1_d)
            b2_sb = consts.tile([H, 1], F32)
            nc.sync.dma_start(out=b2_sb, in_=b2_d)
            b3_sb = consts.tile([O, 1], F32)
            nc.sync.dma_start(out=b3_sb, in_=b3_d)
            idh_sb = consts.tile([128, 128], F16)
            nc.sync.dma_start(out=idh_sb, in_=idh_d)
            idf_sb = consts.tile([128, 128], F32)
            nc.sync.dma_start(out=idf_sb, in_=idf_d)

            for b in range(NBLK):
                r0 = b * IBLK
                # ---- load x block (fp32 -> fp16 cast in DMA) ----
                xs = []
                for s in range(NSUB):
                    xt = xpool.tile([128, D], F16, tag="xnat")
                    nc.gpsimd.dma_start(
                        out=xt, in_=x_d[r0 + s * 128:r0 + (s + 1) * 128, :]
                    )
                    xs.append(xt)

                # ---- per-row stats on DVE; 1/std per 128-col on ACT ----
                mvs = []
                invs = []
                for s in range(NSUB):
                    st6 = spool.tile([128, 6, 6], F32, tag="st6")
                    for k in range(6):
                        nc.vector.bn_stats(
                            out=st6[:, k, :], in_=xs[s][:, k * 512:(k + 1) * 512]
                        )
                    mv = spool.tile([128, 2], F32, tag="mv")
                    nc.vector.bn_aggr(out=mv, in_=st6)
                    mvs.append(mv)
                    inv_col = spool.tile([128, 1], F32, tag="invc")
                    nc.scalar.activation(inv_col, mv[:, 1:2],
                                         AF.Abs_reciprocal_sqrt, scale=DDOF_SCALE)
                    invs.append(inv_col)

                # ---- stats to row layout: [128,1] cols -> [1, 512] psum rows ----
                pmean = pl_pool.tile([1, IBLK], F32, tag="pl")
                pinv = pl_pool.tile([1, IBLK], F32, tag="pl")
                for s in range(NSUB):
                    nc.tensor.transpose(
                        pmean[:, s * 128:(s + 1) * 128], mvs[s][:, 0:1], idf_sb
                    )
                    nc.tensor.transpose(
                        pinv[:, s * 128:(s + 1) * 128], invs[s], idf_sb
                    )
                mean_row = spool.tile([1, IBLK], F16, tag="mrow")
                nc.scalar.copy(mean_row, pmean)
                inv_row = spool.tile([1, IBLK], F32, tag="irow")
                nc.scalar.copy(inv_row, pinv)
                inv_b = spool.tile([H, IBLK], F32, tag="invb")
                nc.gpsimd.partition_broadcast(inv_b, inv_row)

                # ---- transpose x (as regular fp16 matmuls vs identity, to
                # keep the PE HAM-warm) + stream against w1t ----
                py0 = py0_pool.tile([H, IBLK], F32)
                prev = None
                for c2 in range(NCHUNK // 2):
                    pxt = pxt_pool.tile([128, 2 * IBLK], F32)
                    for q in range(2):
                        c = 2 * c2 + q
                        for s in range(NSUB):
                            nc.tensor.matmul(
                                pxt[:, q * IBLK + s * 128:q * IBLK + (s + 1) * 128],
                                xs[s][:, c * 128:(c + 1) * 128],
                                idh_sb,
                                start=True, stop=True,
                            )
                    xts = xtpool.tile([128, 2 * IBLK], F16, tag="xt")
                    nc.scalar.copy(xts, pxt)
                    if prev is not None:
                        pc2, pxts = prev
                        for q in range(2):
                            c = 2 * pc2 + q
                            nc.tensor.matmul(
                                py0, w1t_sb[:, c, :],
                                pxts[:, q * IBLK:(q + 1) * IBLK],
                                start=(c == 0), stop=False,
                            )
                    prev = (c2, xts)
                pc2, pxts = prev
                for q in range(2):
                    c = 2 * pc2 + q
                    nc.tensor.matmul(
                        py0, w1t_sb[:, c, :],
                        pxts[:, q * IBLK:(q + 1) * IBLK],
                        start=False, stop=False,
                    )
                # mean correction: y0 -= rowsum(w1) (x) mean  (K=1 fp32 matmul)
                nc.tensor.matmul(py0, negs_sb, mean_row, start=False, stop=True)

                # ---- normalize + layer 1 activation ----
                t1 = spool.tile([H, IBLK], F32, tag="t1")
                nc.vector.tensor_mul(t1, py0, inv_b)
                h1 = spool.tile([H, IBLK], F16, tag="h1")
                nc.scalar.activation(h1, t1, AF.Prelu, bias=b1_sb, scale=1.0,
                                     alpha=0.01)

                # ---- layers 2 and 3 (small fp32 matmuls) ----
                p2 = pl_pool.tile([H, IBLK], F32, tag="pl")
                nc.tensor.matmul(p2, w2t_sb, h1, start=True, stop=True)
                h2 = spool.tile([H, IBLK], F16, tag="h2")
                nc.scalar.activation(h2, p2, AF.Prelu, bias=b2_sb, scale=1.0,
                                     alpha=0.01)
                p3 = pl_pool.tile([O, IBLK], F32, tag="pl")
                nc.tensor.matmul(p3, w3t_sb, h2, start=True, stop=True)
                y3 = spool.tile([O, IBLK], F32, tag="y3")
                nc.scalar.activation(y3, p3, AF.Prelu, bias=b3_sb, scale=1.0,
                                     alpha=0.01)

                # ---- back to natural layout and store ----
                pout = pl_pool.tile([128, NSUB, O], F32, tag="pl")
                for s in range(NSUB):
                    nc.tensor.transpose(
                        pout[:, s, :],
                        y3[:, s * 128:(s + 1) * 128],
                        idf_sb[0:O, 0:O],
                    )
                out_sb = opool.tile([128, NSUB, O], F32, tag="out")
                nc.vector.tensor_copy(out_sb, pout)
                nc.sync.dma_start(
                    out=y_d[r0:r0 + IBLK, :].rearrange("(s p) c -> p s c", p=128),
                    in_=out_sb,
                )

    nc.compile()
    return nc


def _prep_inputs(x, w1, b1, w2, b2, w3, b3):
    x = np.ascontiguousarray(np.asarray(x, dtype=np.float32))
    w1 = np.asarray(w1, dtype=np.float32)
    w2 = np.asarray(w2, dtype=np.float32)
    w3 = np.asarray(w3, dtype=np.float32)
    b1 = np.asarray(b1, dtype=np.float32)
    b2 = np.asarray(b2, dtype=np.float32)
    b3 = np.asarray(b3, dtype=np.float32)

    common = {
        # [128, 24*32]: partition p holds w1.T[c*128+p, :] for each chunk c
        "w1t": np.ascontiguousarray(
            w1.T.reshape(NCHUNK, 128, H).transpose(1, 0, 2).reshape(128, NCHUNK * H)
        ).astype(np.float16),
        "w2t": np.ascontiguousarray(w2.T).astype(np.float16),
        "w3t": np.ascontiguousarray(w3.T).astype(np.float16),
        "negs": np.ascontiguousarray(
            -w1.astype(np.float64).sum(axis=1, keepdims=True).T
        ).astype(np.float16),
        "b1c": np.ascontiguousarray(b1[:, None]),
        "b2c": np.ascontiguousarray(b2[:, None]),
        "b3c": np.ascontiguousarray(b3[:, None]),
        "idh": np.eye(128, dtype=np.float16),
        "idf": np.eye(128, dtype=np.float32),
    }
    in_maps = []
    for c in range(N_CORES):
        m = dict(common)
        m["x"] = x[c * B_CORE:(c + 1) * B_CORE]
        in_maps.append(m)
    return in_maps


def kernel(x, w1, b1, w2, b2, w3, b3):
    global LAST_EXEC_NS
    if "nc" not in _CACHE:
        _CACHE["nc"] = _build()
    nc = _CACHE["nc"]
    in_maps = _prep_inputs(x, w1, b1, w2, b2, w3, b3)
    trace = bool(int(os.environ.get("KERNEL_PROFILE", "0")))
    res = run_bass_kernel_spmd(nc, in_maps, core_ids=list(range(N_CORES)),
                               trace=trace)
    LAST_EXEC_NS = res.exec_time_ns
    out = np.concatenate([r["y"] for r in res.results], axis=0)
    return out.astype(np.float32)
